# revision 19
# baseline (speedup 1.0000x reference)
"""Trainium2 Bass kernel for a GNN node-aggregator.

Math (reference):
    out[n] = sum_k Linear(concat(v[n], u[k, n]))          with W = [Wv | Wu]
           = (sum_k u[k]) @ Wu.T  +  K * (v @ Wv.T)  +  K * b

The neighbor sum commutes with the linear layer AND with the transpose,
so the kernel computes out.T column blocks directly:

    out.T[:, blk] = sum_k Wu.T.T @ u[k].T[:, blk]  +  (K Wv).T.T @ v.T[:, blk]

The big [K, N, D] tensor is streamed in fp8e3m4 (the harness error gate
is 2e-2; fp8 on the neighbors costs ~4e-3), v and the weights in bf16,
and out.T is written back in bf16.  The host pre-transposes each core's
shard so every 448-node chunk is one contiguous 1.75 MB DMA with
14 KB-per-partition runs.  The K-sum of a chunk is computed one of two
ways, balanced so Tensor and Vector engines are both busy under the
DMA roofline:

 *  PE chunks (layout [d, k, n]): 32 accumulating matmuls with the
    weights stationary — the sum happens in PSUM, no reduction op.
 *  DVE chunks (layout [d, n, k]): one vector tensor_reduce over the
    innermost k axis into fp32, a scalar-engine cast to bf16, then a
    single matmul.

The bias is fused into the scalar-engine PSUM->SBUF copy (Identity
activation with a per-partition bias AP).

Distribution: nodes are sharded across 8 NeuronCores.  Every core runs
the same program over 6272 = 14*448 nodes; the core slices overlap
slightly (50000 is not divisible by 8*448) and the host gather keeps
each core's owned rows only.
"""

import numpy as np

N_NODES = 50000
K_NB = 32
D = 128  # in features
O = 128  # out features
P = 128  # SBUF partitions

N_CORES = 8
CHUNK_N = 448          # nodes per PSUM block (<= 512 f32 per bank)
N_CHUNKS = 14
NC_NODES = CHUNK_N * N_CHUNKS  # 6272 nodes per core (overlapped shard)

N_DV = 4               # chunks whose K-sum runs on the Vector engine


def _dv_chunks(n_dv=N_DV):
    if n_dv == 0:
        return []
    stride = N_CHUNKS / n_dv
    return sorted({int((i + 0.5) * stride) for i in range(n_dv)})


def _core_starts():
    step = N_NODES // N_CORES
    return [min(c * step, N_NODES - NC_NODES) for c in range(N_CORES)]


def _build(
    repeats=1,
    n_dv=N_DV,
    nb_bufs=4,
    out_bufs=3,
    psum_bufs=3,
    nop_cycles=0,
    diag_k=K_NB,
    out_q="scalar",
    load_dual=False,
):
    """Build the per-core Bass program (SPMD: same NEFF on all cores)."""
    import concourse.mybir as mybir
    import concourse.tile as tile
    from concourse import bacc

    f32 = mybir.dt.float32
    bf16 = mybir.dt.bfloat16
    f8 = mybir.dt.float8e3

    dv = _dv_chunks(n_dv)
    pe = [c for c in range(N_CHUNKS) if c not in dv]
    n_pe = len(pe)
    n_dv = len(dv)

    nc = bacc.Bacc(trn_type="TRN2", name="node_aggregator")
    # PE chunks [j, d, k, n]; DVE chunks [j, d, n, k]
    nbp = (
        nc.dram_tensor("nbp", [n_pe, P, K_NB, CHUNK_N], f8, kind="ExternalInput")
        if n_pe
        else None
    )
    nbd = (
        nc.dram_tensor("nbd", [n_dv, P, CHUNK_N, K_NB], f8, kind="ExternalInput")
        if n_dv
        else None
    )
    vtb = nc.dram_tensor("vtb", [P, NC_NODES], bf16, kind="ExternalInput")  # v.T
    wub = nc.dram_tensor("wub", [D, O], bf16, kind="ExternalInput")   # Wu.T
    wvk = nc.dram_tensor("wvk", [D, O], bf16, kind="ExternalInput")   # K*Wv.T
    bk = nc.dram_tensor("bk", [O, 1], f32, kind="ExternalInput")      # K*b
    outT = nc.dram_tensor("outT", [O, NC_NODES], bf16, kind="ExternalOutput")

    ident = mybir.ActivationFunctionType.Identity

    with tile.TileContext(nc) as tc:
        with (
            tc.tile_pool(name="cpool", bufs=1) as cpool,
            tc.tile_pool(name="npool", bufs=nb_bufs) as npool,
            tc.tile_pool(name="ndpool", bufs=2) as ndpool,
            tc.tile_pool(name="spool", bufs=2) as spool,
            tc.tile_pool(name="sbpool", bufs=2) as sbpool,
            tc.tile_pool(name="opool", bufs=min(out_bufs, 2)) as opool,
            tc.tile_pool(name="psp", bufs=psum_bufs, space="PSUM") as psp,
        ):
            # Calibration aid for the test harness: a known-duration delay
            # on the otherwise idle Pool engine, parallel to the kernel.
            rem = nop_cycles
            while rem > 0:
                step = min(rem, 1 << 24)
                nc.gpsimd.nop(cycle_cnt=step, nofuse=True)
                rem -= step

            wub_t = cpool.tile([D, O], bf16)
            nc.sync.dma_start(wub_t[:], wub[:])
            wvk_t = cpool.tile([D, O], bf16)
            nc.sync.dma_start(wvk_t[:], wvk[:])
            bk_t = cpool.tile([O, 1], f32)
            nc.sync.dma_start(bk_t[:], bk[:])
            vt = cpool.tile([P, NC_NODES], bf16)
            nc.sync.dma_start(vt[:], vtb[:])

            pe_pos = {c: j for j, c in enumerate(pe)}
            dv_pos = {c: j for j, c in enumerate(dv)}
            out_eng = {
                "sync": nc.sync,
                "scalar": nc.scalar,
                "gpsimd": nc.gpsimd,
            }[out_q]

            for _ in range(repeats):
                oslab = opool.tile([O, NC_NODES], bf16, tag="oslab")
                for c in range(N_CHUNKS):
                    cs = slice(c * CHUNK_N, (c + 1) * CHUNK_N)
                    load_eng = nc.scalar if (load_dual and c % 2) else nc.sync
                    ps = psp.tile([O, CHUNK_N], f32, tag="ps")
                    if c in dv_pos:
                        nbt = ndpool.tile([P, CHUNK_N, K_NB], f8, tag="nbtd")
                        load_eng.dma_start(nbt[:], nbd[:][dv_pos[c]])
                        st = spool.tile([P, CHUNK_N], f32, tag="st")
                        nc.vector.tensor_reduce(
                            st[:],
                            nbt[:],
                            axis=mybir.AxisListType.X,
                            op=mybir.AluOpType.add,
                        )
                        sb = sbpool.tile([P, CHUNK_N], bf16, tag="sb")
                        nc.scalar.copy(sb[:], st[:])
                        nc.tensor.matmul(
                            ps[:], lhsT=wub_t[:], rhs=sb[:], start=True, stop=False
                        )
                    else:
                        nbt = npool.tile([P, K_NB, CHUNK_N], f8, tag="nbtp")
                        load_eng.dma_start(nbt[:], nbp[:][pe_pos[c]])
                        for k in range(diag_k):
                            nc.tensor.matmul(
                                ps[:],
                                lhsT=wub_t[:],
                                rhs=nbt[:, k, :],
                                start=(k == 0),
                                stop=False,
                            )
                    v_start = c not in dv_pos and diag_k == 0
                    nc.tensor.matmul(
                        ps[:], lhsT=wvk_t[:], rhs=vt[:, cs], start=v_start, stop=True
                    )
                    nc.scalar.activation(oslab[:, cs], ps[:], ident, bias=bk_t[:])
                out_eng.dma_start(outT[:], oslab[:])
    nc.compile()
    return nc


def _prep_inputs(inputs, n_dv=N_DV):
    """Host-side staging: quantize + per-core transpose per chunk type."""
    import ml_dtypes

    f8 = ml_dtypes.float8_e3m4
    bf = ml_dtypes.bfloat16

    v = np.asarray(inputs["v"], dtype=np.float32)
    neighbors = np.asarray(inputs["neighbors"], dtype=np.float32)
    W = np.asarray(inputs["W"], dtype=np.float32)
    b = np.asarray(inputs["b"], dtype=np.float32)

    Wv, Wu = W[:, :D], W[:, D:]
    wub = np.ascontiguousarray(Wu.T).astype(bf)
    wvk = np.ascontiguousarray(Wv.T * np.float32(K_NB)).astype(bf)
    bk = np.ascontiguousarray((np.float32(K_NB) * b)[:, None], dtype=np.float32)

    dv = _dv_chunks(n_dv)
    pe = [c for c in range(N_CHUNKS) if c not in dv]

    q8 = neighbors.astype(f8)  # [K, N, D]
    in_maps = []
    for s in _core_starts():
        x = q8[:, s : s + NC_NODES, :]               # [K, 6272, D]
        x = x.reshape(K_NB, N_CHUNKS, CHUNK_N, D)    # [K, c, n, d]
        m = {
            "vtb": np.ascontiguousarray(v[s : s + NC_NODES].T).astype(bf),
            "wub": wub,
            "wvk": wvk,
            "bk": bk,
        }
        if pe:
            m["nbp"] = np.ascontiguousarray(
                x[:, pe].transpose(1, 3, 0, 2)       # [j, d, K, n]
            )
        if dv:
            m["nbd"] = np.ascontiguousarray(
                x[:, dv].transpose(1, 3, 2, 0)       # [j, d, n, K]
            )
        in_maps.append(m)
    return in_maps


def kernel(v, neighbors, W, b):
    from concourse.bass_utils import run_bass_kernel_spmd

    in_maps = _prep_inputs(
        {"v": v, "neighbors": neighbors, "W": W, "b": b}
    )
    nc = _build()
    res = run_bass_kernel_spmd(nc, in_maps, core_ids=list(range(N_CORES)))

    out = np.empty((N_NODES, O), dtype=np.float32)
    step = N_NODES // N_CORES
    for c, s in enumerate(_core_starts()):
        own_lo = c * step
        own_hi = N_NODES if c == N_CORES - 1 else (c + 1) * step
        r = np.asarray(res.results[c]["outT"])       # [o, n] bf16
        full = r.T.astype(np.float32)
        out[own_lo:own_hi] = full[own_lo - s : own_hi - s]
    return out


# revision 25
# speedup vs baseline: 1.1025x; 1.1025x over previous
"""Trainium2 Bass kernel for a GNN node-aggregator.

Math (reference):
    out[n] = sum_k Linear(concat(v[n], u[k, n]))          with W = [Wv | Wu]
           = (sum_k u[k]) @ Wu.T  +  K * (v @ Wv.T)  +  K * b

The neighbor sum commutes with the linear layer AND with the transpose,
so the kernel computes out.T column blocks directly:

    out.T[:, blk] = sum_k Wu.T.T @ u[k].T[:, blk]  +  (K Wv).T.T @ v.T[:, blk]

The big [K, N, D] tensor is streamed in fp8e3m4 (the harness error gate
is 2e-2; fp8 on the neighbors costs ~4e-3), v and the weights in bf16,
and out.T is written back in bf16.  The host pre-transposes each core's
shard so every 448-node chunk is one contiguous 1.75 MB DMA with
14 KB-per-partition runs.  The K-sum of a chunk is computed one of two
ways, balanced so Tensor and Vector engines are both busy under the
DMA roofline:

 *  PE chunks (layout [d, k, n]): 32 accumulating matmuls with the
    weights stationary — the sum happens in PSUM, no reduction op.
 *  DVE chunks (layout [d, n, k]): one vector tensor_reduce over the
    innermost k axis into fp32, a scalar-engine cast to bf16, then a
    single matmul.

The bias is fused into the scalar-engine PSUM->SBUF copy (Identity
activation with a per-partition bias AP).

Distribution: nodes are sharded across 8 NeuronCores.  Every core runs
the same program over 6272 = 14*448 nodes; the core slices overlap
slightly (50000 is not divisible by 8*448) and the host gather keeps
each core's owned rows only.
"""

import numpy as np

N_NODES = 50000
K_NB = 32
D = 128  # in features
O = 128  # out features
P = 128  # SBUF partitions

N_CORES = 8
CHUNK_N = 448          # nodes per PSUM block (<= 512 f32 per bank)
N_CHUNKS = 14
NC_NODES = CHUNK_N * N_CHUNKS  # 6272 nodes per core (overlapped shard)

N_DV = 4               # chunks whose K-sum runs on the Vector engine


def _dv_chunks(n_dv=N_DV):
    if n_dv == 0:
        return []
    stride = N_CHUNKS / n_dv
    return sorted({int((i + 0.5) * stride) for i in range(n_dv)})


def _core_starts():
    step = N_NODES // N_CORES
    return [min(c * step, N_NODES - NC_NODES) for c in range(N_CORES)]


def _build(
    repeats=1,
    n_dv=N_DV,
    nb_bufs=3,
    out_bufs=3,
    psum_bufs=3,
    nop_cycles=0,
    diag_k=K_NB,
    out_q="scalar",
    load_dual=True,
    pair_loads=True,
    out_split=False,
):
    """Build the per-core Bass program (SPMD: same NEFF on all cores)."""
    import concourse.mybir as mybir
    import concourse.tile as tile
    from concourse import bacc

    f32 = mybir.dt.float32
    bf16 = mybir.dt.bfloat16
    f8 = mybir.dt.float8e3

    dv = _dv_chunks(n_dv)
    pe = [c for c in range(N_CHUNKS) if c not in dv]
    n_pe = len(pe)
    n_dv = len(dv)

    nc = bacc.Bacc(trn_type="TRN2", name="node_aggregator")
    # PE chunks [j, d, k, n]; DVE chunks [j, d, n, k]
    nbp = (
        nc.dram_tensor("nbp", [n_pe, P, K_NB, CHUNK_N], f8, kind="ExternalInput")
        if n_pe
        else None
    )
    nbd = (
        nc.dram_tensor("nbd", [n_dv, P, CHUNK_N, K_NB], f8, kind="ExternalInput")
        if n_dv
        else None
    )
    vtb = nc.dram_tensor("vtb", [P, NC_NODES], bf16, kind="ExternalInput")  # v.T
    wub = nc.dram_tensor("wub", [D, O], bf16, kind="ExternalInput")   # Wu.T
    wvk = nc.dram_tensor("wvk", [D, O], bf16, kind="ExternalInput")   # K*Wv.T
    bk = nc.dram_tensor("bk", [O, 1], f32, kind="ExternalInput")      # K*b
    outT = nc.dram_tensor("outT", [O, NC_NODES], bf16, kind="ExternalOutput")

    ident = mybir.ActivationFunctionType.Identity

    with tile.TileContext(nc) as tc:
        with (
            tc.tile_pool(name="cpool", bufs=1) as cpool,
            tc.tile_pool(name="npool", bufs=nb_bufs) as npool,
            tc.tile_pool(name="ndpool", bufs=2) as ndpool,
            tc.tile_pool(name="spool", bufs=2) as spool,
            tc.tile_pool(name="sbpool", bufs=2) as sbpool,
            tc.tile_pool(name="opool", bufs=min(out_bufs, 2)) as opool,
            tc.tile_pool(name="psp", bufs=psum_bufs, space="PSUM") as psp,
        ):
            # Calibration aid for the test harness: a known-duration delay
            # on the otherwise idle Pool engine, parallel to the kernel.
            rem = nop_cycles
            while rem > 0:
                step = min(rem, 1 << 24)
                nc.gpsimd.nop(cycle_cnt=step, nofuse=True)
                rem -= step

            wub_t = cpool.tile([D, O], bf16)
            nc.sync.dma_start(wub_t[:], wub[:])
            wvk_t = cpool.tile([D, O], bf16)
            nc.sync.dma_start(wvk_t[:], wvk[:])
            bk_t = cpool.tile([O, 1], f32)
            nc.sync.dma_start(bk_t[:], bk[:])
            vt = cpool.tile([P, NC_NODES], bf16)
            nc.sync.dma_start(vt[:], vtb[:])

            pe_pos = {c: j for j, c in enumerate(pe)}
            dv_pos = {c: j for j, c in enumerate(dv)}
            out_eng = {
                "sync": nc.sync,
                "scalar": nc.scalar,
                "gpsimd": nc.gpsimd,
            }[out_q]
            nbp_r = nbp[:].rearrange("j p k n -> p j k n") if n_pe else None
            nbd_r = nbd[:].rearrange("j p n k -> p j n k") if n_dv else None

            if pair_loads:
                assert n_pe % 2 == 0 and n_dv % 2 == 0
                pe_groups = [(pe[i], pe[i + 1]) for i in range(0, n_pe, 2)]
                dv_groups = [(dv[i], dv[i + 1]) for i in range(0, n_dv, 2)]
            else:
                pe_groups = [(c,) for c in pe]
                dv_groups = [(c,) for c in dv]
            groups = sorted(pe_groups + dv_groups, key=lambda g: g[0])

            for _ in range(repeats):
                oslab = opool.tile([O, NC_NODES], bf16, tag="oslab")
                for gi, g in enumerate(groups):
                    is_dv = g[0] in dv_pos
                    load_eng = nc.scalar if (load_dual and gi % 2) else nc.sync
                    w = len(g)
                    if is_dv:
                        nbt = ndpool.tile([P, w, CHUNK_N, K_NB], f8, tag="nbtd")
                        j0 = dv_pos[g[0]]
                        load_eng.dma_start(nbt[:], nbd_r[:, j0 : j0 + w])
                        st = spool.tile([P, w, CHUNK_N], f32, tag="st")
                        nc.vector.tensor_reduce(
                            st[:],
                            nbt[:],
                            axis=mybir.AxisListType.X,
                            op=mybir.AluOpType.add,
                        )
                        sb = sbpool.tile([P, w, CHUNK_N], bf16, tag="sb")
                        nc.scalar.copy(sb[:], st[:])
                    else:
                        nbt = npool.tile([P, w, K_NB, CHUNK_N], f8, tag="nbtp")
                        j0 = pe_pos[g[0]]
                        load_eng.dma_start(nbt[:], nbp_r[:, j0 : j0 + w])
                    for h, c in enumerate(g):
                        cs = slice(c * CHUNK_N, (c + 1) * CHUNK_N)
                        ps = psp.tile([O, CHUNK_N], f32, tag="ps")
                        if is_dv:
                            nc.tensor.matmul(
                                ps[:],
                                lhsT=wub_t[:],
                                rhs=sb[:, h, :],
                                start=True,
                                stop=False,
                            )
                        else:
                            for k in range(diag_k):
                                nc.tensor.matmul(
                                    ps[:],
                                    lhsT=wub_t[:],
                                    rhs=nbt[:, h, k, :],
                                    start=(k == 0),
                                    stop=False,
                                )
                        v_start = (not is_dv) and diag_k == 0
                        nc.tensor.matmul(
                            ps[:],
                            lhsT=wvk_t[:],
                            rhs=vt[:, cs],
                            start=v_start,
                            stop=True,
                        )
                        nc.scalar.activation(
                            oslab[:, cs], ps[:], ident, bias=bk_t[:]
                        )
                    if out_split and g[-1] == N_CHUNKS // 2:
                        half = (N_CHUNKS // 2 + 1) * CHUNK_N
                        out_eng.dma_start(outT[:, :half], oslab[:, :half])
                if out_split:
                    half = (N_CHUNKS // 2 + 1) * CHUNK_N
                    out_eng.dma_start(outT[:, half:], oslab[:, half:])
                else:
                    out_eng.dma_start(outT[:], oslab[:])
    nc.compile()
    return nc


def _prep_inputs(inputs, n_dv=N_DV):
    """Host-side staging: quantize + per-core transpose per chunk type."""
    import ml_dtypes

    f8 = ml_dtypes.float8_e3m4
    bf = ml_dtypes.bfloat16

    v = np.asarray(inputs["v"], dtype=np.float32)
    neighbors = np.asarray(inputs["neighbors"], dtype=np.float32)
    W = np.asarray(inputs["W"], dtype=np.float32)
    b = np.asarray(inputs["b"], dtype=np.float32)

    Wv, Wu = W[:, :D], W[:, D:]
    wub = np.ascontiguousarray(Wu.T).astype(bf)
    wvk = np.ascontiguousarray(Wv.T * np.float32(K_NB)).astype(bf)
    bk = np.ascontiguousarray((np.float32(K_NB) * b)[:, None], dtype=np.float32)

    dv = _dv_chunks(n_dv)
    pe = [c for c in range(N_CHUNKS) if c not in dv]

    q8 = neighbors.astype(f8)  # [K, N, D]
    in_maps = []
    for s in _core_starts():
        x = q8[:, s : s + NC_NODES, :]               # [K, 6272, D]
        x = x.reshape(K_NB, N_CHUNKS, CHUNK_N, D)    # [K, c, n, d]
        m = {
            "vtb": np.ascontiguousarray(v[s : s + NC_NODES].T).astype(bf),
            "wub": wub,
            "wvk": wvk,
            "bk": bk,
        }
        if pe:
            m["nbp"] = np.ascontiguousarray(
                x[:, pe].transpose(1, 3, 0, 2)       # [j, d, K, n]
            )
        if dv:
            m["nbd"] = np.ascontiguousarray(
                x[:, dv].transpose(1, 3, 2, 0)       # [j, d, n, K]
            )
        in_maps.append(m)
    return in_maps


def kernel(v, neighbors, W, b):
    from concourse.bass_utils import run_bass_kernel_spmd

    in_maps = _prep_inputs(
        {"v": v, "neighbors": neighbors, "W": W, "b": b}
    )
    nc = _build()
    res = run_bass_kernel_spmd(nc, in_maps, core_ids=list(range(N_CORES)))

    out = np.empty((N_NODES, O), dtype=np.float32)
    step = N_NODES // N_CORES
    for c, s in enumerate(_core_starts()):
        own_lo = c * step
        own_hi = N_NODES if c == N_CORES - 1 else (c + 1) * step
        r = np.asarray(res.results[c]["outT"])       # [o, n] bf16
        full = r.T.astype(np.float32)
        out[own_lo:own_hi] = full[own_lo - s : own_hi - s]
    return out
